# revision 17
# baseline (speedup 1.0000x reference)
"""MoE experts (32 experts, top-2, SwiGLU MLP) on 8 trn2 NeuronCores.

Expert-parallel sharding: core c owns 4 experts. Routing metadata
(Switch-style positions / per-expert slot lists) is computed on host from
top_k_indices; each core receives one "mega" input tensor holding, in
exact consumption order, each expert's dispatched activations and
pre-transposed weights. The device runs the grouped SwiGLU MLP +
routing-weight scaling (fp16 operands, fp32 accumulation) and returns
per-slot outputs (fp16); host scatters them back to (token, k) and sums
over the top-k axis and cores.

Device-side schedule notes (from trace analysis of the 65.45us baseline):
- All loads stream on the SP HWDGE ring as a few big chunks in
  consumption order (desc-gen is ~0.65us/DMA on the issuing engine, so
  fewer+bigger is better); w1 is laid out in o-major 512-col blocks
  ordered (gate_it, up_it) so the first matmul group only needs the
  first chunk.
- The PE is warmed with dummy matmuls during the initial DMA head so the
  HAM clock-gate (1.2 -> 2.4 GHz after ~3.4us of sustained PE activity)
  un-throttles before the real matmul stream begins.
- Stores are fp16 and go to the SWDGE (gpsimd) ring except the last
  expert's, which are split and issued on the ACT ring for a short tail.
"""

import sys
import types

import numpy as np

# Model dims (hardcoded per problem spec nn_MoEExperts_27109833572673)
T, TOPK, E, H, I = 4096, 2, 32, 512, 1024
CAP = 2 * (T * TOPK) // E  # 512
NCORES = 8
EPC = E // NCORES  # experts per core = 4
HT = H // 128  # 4 h-tiles
IT = I // 128  # 8 i-tiles

W1COLS = 16 * 512  # 8192: 16 o-major blocks of [HT x 128]
W2COLS = IT * H  # 4096
# w1 block order: (gate it, up it) pairs; gate rows live in o-blocks 0-7,
# up rows in o-blocks 8-15 of the [2I, H] gate_up matrix.
W1_BLOCK_ORDER = [0, 8, 1, 9, 2, 10, 3, 11, 4, 12, 5, 13, 6, 14, 7, 15]

NWARM = 34  # PE warm-up matmuls (N=128) during the DMA head

LAST_RESULTS = None  # BassKernelResults of the most recent device run


def _ensure_profile_hook():
    """Register the NTFF profile hook if the env lacks antenv.axon_hooks.

    Only needed when tracing (BASS_TRACE=1 / trace=True); safe no-op
    otherwise. Mirrors trn_agent_boot.trn_boot step 6.
    """
    try:
        if "antenv.axon_hooks" in sys.modules:
            return
        import antenv

        mod = types.ModuleType("antenv.axon_hooks")
        state = {"hook": None}
        mod.set_axon_ntff_profile_hook = lambda h: state.__setitem__("hook", h)
        mod.get_axon_ntff_profile_hook = lambda: state["hook"]
        sys.modules["antenv.axon_hooks"] = mod
        antenv.axon_hooks = mod
        try:
            from trn_agent_boot.trn_boot import _ntff_profile_via_ctypes

            mod.set_axon_ntff_profile_hook(
                _ntff_profile_via_ctypes("/opt/axon/libaxon_pjrt.so")
            )
        except Exception:
            pass
    except Exception:
        pass


def _routing(top_k_indices, top_k_weights):
    """Per-expert slot lists (ascending flat order == Switch dispatch pos),
    clipped at CAP exactly like the reference's capacity drop."""
    e_flat = np.asarray(top_k_indices).reshape(-1).astype(np.int32)
    w_flat = np.asarray(top_k_weights).reshape(-1).astype(np.float32)
    tok = np.arange(T * TOPK, dtype=np.int32) // TOPK
    order = np.argsort(e_flat, kind="stable")
    sorted_e = e_flat[order]
    starts = np.searchsorted(sorted_e, np.arange(E + 1))
    slots_per_e = [order[starts[e] : starts[e + 1]][:CAP] for e in range(E)]
    return e_flat, w_flat, tok, slots_per_e


_prog_cache = {}


def _build_program(m_pads):
    """One SPMD program: per-core grouped SwiGLU MLP over EPC experts,
    position j padded to m_pads[j] slots."""
    import concourse.bacc as bacc
    import concourse.mybir as mybir
    from concourse.tile import TileContext

    f32 = mybir.dt.float32
    f16 = mybir.dt.float16
    mmdt = f16
    offs = [0]
    for m in m_pads:
        offs.append(offs[-1] + int(m))
    slots = offs[-1]
    megacols = [HT * int(m) + W1COLS + W2COLS for m in m_pads]
    megaoffs = [0]
    for mc in megacols:
        megaoffs.append(megaoffs[-1] + mc)

    nc = bacc.Bacc("TRN2", target_bir_lowering=False, debug=False,
                   num_devices=NCORES)
    mega_d = nc.declare_dram_parameter("mega", [128, megaoffs[-1]], mmdt,
                                       isOutput=False)
    wsc_d = nc.declare_dram_parameter("wsc", [1, slots], f32, isOutput=False)
    y_d = nc.declare_dram_parameter("y", [128, HT * slots], f16,
                                    isOutput=True)

    with TileContext(nc) as tc:
        with (
            tc.tile_pool(name="mega", bufs=1) as megap,
            tc.tile_pool(name="act", bufs=3) as actp,
            tc.tile_pool(name="ps1", bufs=3, space="PSUM") as ps1p,
            tc.tile_pool(name="ps2", bufs=2, space="PSUM") as ps2p,
            tc.tile_pool(name="outp", bufs=2) as outp,
            tc.tile_pool(name="misc", bufs=1) as miscp,
        ):
            # --- PE warm-up: keep the PE busy during the DMA head so the
            # HAM clock-gate reaches 8/8 before the real matmul stream.
            wtile = miscp.tile([128, 128], mmdt, tag="warm")
            nc.gpsimd.memset(wtile[:], 0.0)
            warm_ps = ps1p.tile([128, int(m_pads[0])], f32, tag="pg",
                                name="pg")
            for _ in range(NWARM):
                nc.tensor.matmul(warm_ps[:, :128], wtile[:], wtile[:],
                                 start=True, stop=True)

            wsc_t = miscp.tile([1, slots], f32, tag="wsc")
            nc.gpsimd.dma_start(out=wsc_t[:], in_=wsc_d[:])
            # Routing-weight rows for the column scale after mm2; all four
            # broadcasts up front so later gpsimd store desc-gens can't
            # delay them.
            wrows = []
            for j in range(EPC):
                m = int(m_pads[j])
                wr = miscp.tile([128, m], f32, tag=f"wrow{j}",
                                name=f"wrow{j}")
                nc.gpsimd.partition_broadcast(
                    wr[:], wsc_t[0:1, offs[j] : offs[j] + m])
                wrows.append(wr)

            for j in range(EPC):
                m = int(m_pads[j])
                base = megaoffs[j]
                ncols = megacols[j]
                # Per-expert layout: [block0 (512) | xd (HT*m) | blocks
                # 1..15 | w2]. Block 0 leads so the very first matmul
                # group's chunk (block0 + xd_ht0) is only ~164KB.
                XW = 512 + HT * m  # start of w1 blocks 1..15
                W2O = HT * m + W1COLS
                mg = megap.tile([128, ncols], mmdt, tag=f"mega{j}",
                                name=f"mega{j}")

                def w1s(b, ht, mg=mg, XW=XW):
                    if b == 0:
                        return mg[:, ht * 128 : (ht + 1) * 128]
                    o0 = XW + (b - 1) * 512 + ht * 128
                    return mg[:, o0 : o0 + 128]

                # Loads: consumption-ordered chunks on the SP ring (the
                # Sync engine is otherwise idle, so its ~0.65us/desc-gen
                # runs far ahead of the wire). Chunk grain is set by the
                # warm-stream consumption rate (a 512-col w1 block every
                # ~0.47us = ~280 GB/s) vs the ring's ramping delivery plus
                # the ~0.8us completion-sem latency a chunk's first column
                # pays: expert 0 gets fine chunks, the rest get coarser
                # ones as the ring builds a lead.
                if j == 0:
                    # B0+xd_ht0 | xd rest | B1..B5 singly (they gate the
                    # first warm groups, whose deadlines are ~0.47us apart)
                    # | B6..B15 in pairs | w2 halves
                    cuts = [0, 512 + m]
                    cuts += [XW + 512 * k for k in range(6)]
                    cuts += [XW + 2560 + 1024 * (k + 1) for k in range(5)]
                    cuts += [W2O + 2048, ncols]
                elif j < EPC - 1:
                    cuts = [0, XW + 2560, XW + 5632, W2O, ncols]
                else:
                    cuts = [0, XW + 3584, W2O, ncols]
                for c0, c1 in zip(cuts[:-1], cuts[1:]):
                    nc.sync.dma_start(out=mg[:, c0:c1],
                                      in_=mega_d[:, base + c0 : base + c1])

                xd = [mg[:, 512 + ht * m : 512 + (ht + 1) * m]
                      for ht in range(HT)]

                # mm1: pg/pu[o, s] += W1[o, h-tile] @ xd[h-tile, s]
                acts = []
                for it in range(IT):
                    pg = ps1p.tile([128, m], f32, tag="pg", name="pg")
                    pu = ps1p.tile([128, m], f32, tag="pu", name="pu")
                    for ht in range(HT):
                        nc.tensor.matmul(pg[:], w1s(2 * it, ht), xd[ht],
                                         start=(ht == 0), stop=(ht == HT - 1))
                    for ht in range(HT):
                        nc.tensor.matmul(pu[:], w1s(2 * it + 1, ht), xd[ht],
                                         start=(ht == 0), stop=(ht == HT - 1))
                    sg = actp.tile([128, m], f32, tag="sg", name="sg")
                    nc.scalar.activation(sg[:], pg[:],
                                         mybir.ActivationFunctionType.Silu)
                    a = actp.tile([128, m], mmdt, tag=f"a{it}", name=f"a{it}")
                    nc.vector.tensor_mul(a[:], sg[:], pu[:])
                    acts.append(a)

                # mm2: y^T[h, s] = W2[h, i] @ act[i, s]; routing weight is a
                # column scale via the partition-broadcast weight row.
                ot = outp.tile([128, HT * m], f16, tag="ot", name="ot")
                for ht2 in range(HT):
                    ps2 = ps2p.tile([128, m], f32, tag="ps2", name="ps2")
                    for it in range(IT):
                        o0 = W2O + it * H + ht2 * 128
                        nc.tensor.matmul(ps2[:], mg[:, o0 : o0 + 128],
                                         acts[it],
                                         start=(it == 0), stop=(it == IT - 1))
                    nc.vector.tensor_mul(
                        ot[:, ht2 * m : (ht2 + 1) * m], ps2[:], wrows[j][:])
                    if j == EPC - 1:
                        # last expert: ship each slice as soon as its mul
                        # lands so only one [128, m] store trails the final
                        # matmul group
                        nc.scalar.dma_start(
                            out=y_d[:, HT * offs[j] + ht2 * m :
                                    HT * offs[j] + (ht2 + 1) * m],
                            in_=ot[:, ht2 * m : (ht2 + 1) * m])
                if j < EPC - 1:
                    nc.gpsimd.dma_start(
                        out=y_d[:, HT * offs[j] : HT * offs[j + 1]],
                        in_=ot[:])

    nc.finalize()
    return nc


def _pack_core(hs, gup, dwn, exps, m_pads, offs, slots_per_e, tok, w_flat):
    """Build one core's mega tensor (fp16) + wsc row (f32)."""
    slots = offs[-1]
    parts = []
    wsc = np.zeros((1, slots), np.float32)
    for j, e in enumerate(exps):
        m = m_pads[j]
        sl = slots_per_e[e]
        wsc[0, offs[j] : offs[j] + len(sl)] = w_flat[sl]
        xdblk = np.zeros((m, H), np.float32)
        xdblk[: len(sl)] = hs[tok[sl]]
        # w1m[p, b*512 + ht*128 + o8] = gate_up[e, O(b)*128 + o8, ht*128 + p]
        g = gup[e].reshape(16, 128, HT, 128)  # [O, o8, ht, p]
        w1m = g[W1_BLOCK_ORDER].transpose(3, 0, 2, 1).reshape(128, W1COLS)
        # layout: [block0 | xd | blocks 1..15 | w2]
        parts.append(w1m[:, :512])
        # xdT[p, ht*m + s] = xd[s, ht*128 + p]
        parts.append(xdblk.reshape(m, HT, 128).transpose(2, 1, 0)
                     .reshape(128, HT * m))
        parts.append(w1m[:, 512:])
        # w2m[p, it*H + h] = down[e, h, it*128 + p]
        parts.append(dwn[e].reshape(H, IT, 128).transpose(2, 1, 0)
                     .reshape(128, W2COLS))
    mega = np.ascontiguousarray(np.concatenate(parts, axis=1)
                                .astype(np.float16))
    return mega, wsc


def kernel(hidden_states, top_k_indices, top_k_weights, gate_up_proj,
           down_proj):
    global LAST_RESULTS
    _ensure_profile_hook()
    from concourse.bass_utils import run_bass_kernel_spmd

    hs = np.ascontiguousarray(np.asarray(hidden_states, dtype=np.float32))
    gup = np.asarray(gate_up_proj, dtype=np.float32)
    dwn = np.asarray(down_proj, dtype=np.float32)

    e_flat, w_flat, tok, slots_per_e = _routing(top_k_indices, top_k_weights)
    counts = np.array([len(s) for s in slots_per_e])
    # Load-balance: sort experts by routed count and deal them out in
    # rounds of NCORES — position j on every core handles one expert from
    # round j, so the per-position compile-time pad (the round max) stays
    # as tight as possible.
    sorted_eids = np.argsort(-counts, kind="stable")
    assign = sorted_eids.reshape(EPC, NCORES)  # [position, core]
    m_pads = tuple(
        int(min(CAP, max(128, ((int(counts[assign[j]].max()) + 3) // 4) * 4)))
        for j in range(EPC))
    offs = [0]
    for m in m_pads:
        offs.append(offs[-1] + m)

    if m_pads not in _prog_cache:
        _prog_cache[m_pads] = _build_program(m_pads)
    nc = _prog_cache[m_pads]

    in_maps = []
    core_exps = []
    for c in range(NCORES):
        exps = [int(assign[j, c]) for j in range(EPC)]
        core_exps.append(exps)
        mega, wsc = _pack_core(hs, gup, dwn, exps, m_pads, offs,
                               slots_per_e, tok, w_flat)
        in_maps.append({"mega": mega, "wsc": wsc})

    res = run_bass_kernel_spmd(nc, in_maps, list(range(NCORES)))
    LAST_RESULTS = res

    # Combine: scatter per-slot outputs back to flat (token, k) slots and
    # reduce over the top-k axis and cores.
    y_tk = np.zeros((T * TOPK, H), np.float32)
    for c in range(NCORES):
        yc = res.results[c]["y"].astype(np.float32)  # [128, HT*slots]
        for j, e in enumerate(core_exps[c]):
            sl = slots_per_e[e]
            blk = (yc[:, HT * offs[j] : HT * offs[j + 1]]
                   .reshape(128, HT, m_pads[j]))
            # y[s, h] with h = ht*128 + p
            y_tk[sl] = blk.transpose(2, 1, 0).reshape(m_pads[j], H)[: len(sl)]
    return y_tk.reshape(T, TOPK, H).sum(axis=1)
